# revision 27
# baseline (speedup 1.0000x reference)
"""Trainium2 Bass kernel: 3-layer GNN message passing (atom embedding).

Data-parallel over the B*N=400000 point axis across 8 NeuronCores.

Numerics: layer 2's GroupNorm has near-degenerate groups (min var ~6e-7
<< eps=1e-5), so any absolute error upstream is amplified ~300x into the
output; plain bf16/fp16/fp32r matmuls all fail the 2e-2 gate (measured).
This kernel keeps fp32-grade accuracy while running the big matmul at
1 cycle/row (vs 4 for fp32) using exact fp16 hi/lo splits:

- Every input value x is split (on host) into hi = fp16(x) and
  lo = fp16(x - hi); fp16 subnormals are honored by the PE (measured:
  3-term hi/lo matmul reaches rel 2e-7), so no scaling is needed.
- mm1 per neighbor-half is TWO fp16 matmul passes into one PSUM tile:
    pass0: [Ahi; Alo] @ [Whi; Whi]  = A @ Whi   (A exact)
    pass1: [Ahi; Alo] @ [Wlo; 0  ]  = Ahi @ Wlo (correction)
  Dropped term Alo@Wlo ~ 2^-24. End-to-end rel err vs reference:
  1.5e-4 (simulated), 100x inside the gate.
- F tile per half: [125, W] fp16 = 48 atom_hi, 48 atom_lo, 8 dist_hi,
  8 dist_lo, 1 ones, 6 emb_hi, 6 emb_lo. Layer 0 folds emb (==1) into
  the ones row and reads only rows 0:113. lhsT pass0 col 104 puts 0.5
  on the ones row, so Prelu(z)[104] == 0.5 per half and the half-summed
  Hs row 104 == 1.0 - a free ones row for mm2's bias.
- mm2 is ONE fp32 matmul (4 cyc/row): lhsT [105, 6] = W2@C replicated
  over the 8 neighbor blocks + row 104 = C@(16 b2), where C is the
  GroupNorm centering matrix, so msg PSUM is already the centered d
  with bias included.
- Engine balance per tile: PE 4 fp16 passes + 1 fp32 (mm2, lag 2);
  ScalarE does both Prelu evacs (DVE cannot read 2 PSUM operands); DVE
  sums the halves (Hs = H0 + H1) and d-copies msg into the c-major etb
  strip; GPSIMD accumulates E.
- GroupNorm batched over GB=14 tiles in c-major [84, T] layout (gather
  DMA from etb on the SWDGE ring), stats matmul in fp32, emb refresh
  writes fp16 hi/lo pairs into both halves' F tiles.
- DMA-instruction parallelism is the scarce resource on this part
  (single big DMA ~34GB/s): each 1.6MB ad load is split into 8 row
  chunks alternating across both HWDGE rings; gather/store and half the
  refreshes ride the SWDGE (gpsimd) ring. This alone was worth ~25%.
- Batches run in software-pipelined even/odd pairs; each batch's GN
  tail is spread hop-by-hop (var/rstd/y/upd/E/refresh) across the
  partner block's tile stream so no engine queue ever blocks on a
  cross-engine chain.
"""
import os
import sys

sys.path.insert(0, "/opt/trn_rl_repo")

import numpy as np

# timing-bisect knobs (correctness is broken when set; bench only)
BENCH_SKIP_GN = bool(int(os.environ.get("BENCH_SKIP_GN", "0")))
BENCH_SKIP_MM = bool(int(os.environ.get("BENCH_SKIP_MM", "0")))

D = 6
K = 16
N_LAYERS = 3
C_IN = 13
EPS = 1e-5
SLOPE = 0.2

N_CORES = 8
T = 512            # points per tile (PSUM bank = 512 fp32)
GB = 14            # point-tiles per groupnorm batch
W = GB * T         # 7168 points per batch
NB = 7             # batches per core
PC = 50000         # points per core
PP = NB * W        # padded points per core = 50176
SROWS = 6 * GB     # 84

F_ROWS = 125       # 48+48+8+8+1 (hbm) + 6 emb_hi + 6 emb_lo (device)
AD_ROWS = 113      # rows loaded from HBM per half
ZROWS = 105        # mm1 out: 8 nbr x 13 + 1 ones


def _f16(x):
    return np.float16(x)


def _split16(x):
    hi = np.float16(x)
    lo = np.float16(x.astype(np.float32) - hi.astype(np.float32))
    return hi, lo


def _pack_weights(W1, b1, W2, b2, gw, gb):
    """Packed lhsT / const tensors (host side, a few KB)."""
    C = np.eye(D, dtype=np.float32) - np.kron(
        np.eye(2, dtype=np.float32), np.ones((3, 3), np.float32) / 3.0)

    # mm1 lhsT: [125, 3 layers * 2 halves * 2 passes * 105] fp16
    l1 = np.zeros((N_LAYERS, 2, 2, F_ROWS, ZROWS), np.float16)
    for i in range(N_LAYERS):
        Wa = W1[i, 6:12, :]        # atom rows [6,13]
        Wd = W1[i, 12:13, :]       # dist row  [1,13]
        We = W1[i, 0:6, :]         # emb rows  [6,13]
        bias = b1[i] + (We.sum(axis=0) if i == 0 else 0.0)
        Wa_h, Wa_l = _split16(Wa)
        Wd_h, Wd_l = _split16(Wd)
        We_h, We_l = _split16(We)
        b_h, b_l = _split16(bias)
        for h in range(2):
            for k8 in range(8):
                cols = slice(k8 * 13, k8 * 13 + 13)
                # pass0: A @ Whi  (hi AND lo rows carry Whi)
                L = l1[i, h, 0]
                L[k8 * 6:(k8 + 1) * 6, cols] = Wa_h
                L[48 + k8 * 6:48 + (k8 + 1) * 6, cols] = Wa_h
                L[96 + k8, cols] = Wd_h
                L[104 + k8, cols] = Wd_h
                L[112, cols] = b_h
                if i > 0:
                    L[113:119, cols] = We_h
                    L[119:125, cols] = We_h
                # pass1: Ahi @ Wlo  (hi rows only)
                L = l1[i, h, 1]
                L[k8 * 6:(k8 + 1) * 6, cols] = Wa_l
                L[96 + k8, cols] = Wd_l
                L[112, cols] = b_l
                if i > 0:
                    L[113:119, cols] = We_l
            # ones output column for mm2 bias: Prelu(0.5)+Prelu(0.5)=1
            l1[i, h, 0][112, 104] = np.float16(0.5)
    l1_flat = np.ascontiguousarray(
        np.concatenate([l1[i, h, p] for i in range(N_LAYERS)
                        for h in range(2) for p in range(2)], axis=1))

    # mm2 lhsT: [105, 18] fp32; row 104 = centered bias
    l2 = np.zeros((ZROWS, N_LAYERS * D), np.float32)
    for i in range(N_LAYERS):
        W2C = (W2[i] @ C).astype(np.float32)
        for k8 in range(8):
            l2[k8 * 13:k8 * 13 + 13, i * D:(i + 1) * D] = W2C
        l2[104, i * D:(i + 1) * D] = C @ (16.0 * b2[i])

    # c-major GroupNorm averaging matrix: p = c*GB + j
    G1 = np.zeros((SROWS, SROWS), np.float32)
    for p in range(SROWS):
        for q in range(SROWS):
            if p % GB == q % GB and (p // GB) // 3 == (q // GB) // 3:
                G1[p, q] = 1.0 / 3.0

    cstg = np.zeros((SROWS, 2 * N_LAYERS), np.float32)
    for p in range(SROWS):
        c = p // GB
        for i in range(N_LAYERS):
            cstg[p, i] = gw[i, c]
            cstg[p, 3 + i] = gb[i, c]
    return l1_flat, l2, G1, cstg


def _build_nc():
    import concourse.bass as bass
    import concourse.bacc as bacc
    import concourse.mybir as mybir
    from concourse import tile

    F32 = mybir.dt.float32
    F16 = mybir.dt.float16
    AF = mybir.ActivationFunctionType
    OP = mybir.AluOpType

    import time as _time
    print(f"[kernel] build start {_time.time():.1f}", flush=True)
    nc = bacc.Bacc("TRN2", target_bir_lowering=False)
    ad0_e = nc.declare_dram_parameter("ad0", [AD_ROWS, PP], F16, isOutput=False)
    ad1_e = nc.declare_dram_parameter("ad1", [AD_ROWS, PP], F16, isOutput=False)
    l1_e = nc.declare_dram_parameter("lhsT1", [F_ROWS, 12 * ZROWS], F16,
                                     isOutput=False)
    l2_e = nc.declare_dram_parameter("lhsT2", [ZROWS, N_LAYERS * D], F32,
                                     isOutput=False)
    g1_e = nc.declare_dram_parameter("g1", [SROWS, SROWS], F32, isOutput=False)
    cg_e = nc.declare_dram_parameter("cstg", [SROWS, 2 * N_LAYERS], F32,
                                     isOutput=False)
    out_e = nc.declare_dram_parameter("out", [D, PP], F32, isOutput=True)

    with tile.TileContext(nc) as tc:
        with tc.tile_pool(name="w", bufs=1) as wp, \
             tc.tile_pool(name="f0", bufs=2) as fp0, \
             tc.tile_pool(name="f1", bufs=2) as fp1, \
             tc.tile_pool(name="e", bufs=2) as ep, \
             tc.tile_pool(name="h", bufs=4) as hp, \
             tc.tile_pool(name="hs", bufs=4) as hsp, \
             tc.tile_pool(name="g", bufs=3) as gp, \
             tc.tile_pool(name="z", bufs=5, space="PSUM") as zp, \
             tc.tile_pool(name="m", bufs=2, space="PSUM") as mp, \
             tc.tile_pool(name="s", bufs=1, space="PSUM") as sp:
            l1 = wp.tile([F_ROWS, 12 * ZROWS], F16)
            l2 = wp.tile([ZROWS, N_LAYERS * D], F32)
            g1 = wp.tile([SROWS, SROWS], F32)
            cg = wp.tile([SROWS, 2 * N_LAYERS], F32)
            eps = wp.tile([128, 1], F32)
            dmy = wp.tile([D, T], F32)
            nc.gpsimd.memset(dmy[:], 0.01)
            nc.sync.dma_start(out=l1[:], in_=l1_e[:])
            nc.sync.dma_start(out=l2[:], in_=l2_e[:])
            nc.sync.dma_start(out=g1[:], in_=g1_e[:])
            nc.sync.dma_start(out=cg[:], in_=cg_e[:])
            nc.gpsimd.memset(eps[:], EPS)

            def lb(i, h, p):
                off = ((i * 2 + h) * 2 + p) * ZROWS
                return l1[:, off:off + ZROWS]

            def emit_load(b):
                # each load split into row chunks across both HWDGE rings:
                # more DMA instructions in flight -> more engine parallelism
                F0 = fp0.tile([F_ROWS, W], F16, tag="F0", name="F0")
                F1 = fp1.tile([F_ROWS, W], F16, tag="F1", name="F1")
                cuts = [0, 15, 29, 43, 57, 71, 85, 99, AD_ROWS]
                for k in range(8):
                    r0, r1 = cuts[k], cuts[k + 1]
                    if k == 7:
                        # one chunk per half rides the idle SWDGE ring
                        nc.gpsimd.dma_start(
                            out=F0[r0:r1, :],
                            in_=ad0_e[r0:r1, b * W:(b + 1) * W])
                        nc.gpsimd.dma_start(
                            out=F1[r0:r1, :],
                            in_=ad1_e[r0:r1, b * W:(b + 1) * W])
                        continue
                    ring = nc.sync if k % 2 == 0 else nc.scalar
                    ring.dma_start(out=F0[r0:r1, :],
                                   in_=ad0_e[r0:r1, b * W:(b + 1) * W])
                    ring2 = nc.scalar if k % 2 == 0 else nc.sync
                    ring2.dma_start(out=F1[r0:r1, :],
                                    in_=ad1_e[r0:r1, b * W:(b + 1) * W])
                return (F0, F1)

            def emit_tiles(Fs2, etb, i, after2=None):
                """after2: dict j -> list of callbacks injected after tile
                j's emission (spread GN-tail hops across the stream)."""
                F0, F1 = Fs2
                fr = AD_ROWS if (i == 0 or BENCH_SKIP_GN) else F_ROWS
                l2s = l2[:, i * D:(i + 1) * D]
                after2 = after2 or {}
                pend = []

                def inject(j):
                    for cb in after2.get(j, ()):
                        cb()

                if BENCH_SKIP_MM:
                    for j in range(GB):
                        nc.vector.tensor_copy(etb[:, j * T:(j + 1) * T],
                                              dmy[:])
                        inject(j)
                    return

                def mm2_stage(j, Hs):
                    msg = mp.tile([D, T], F32, tag="msg", name="msg")
                    nc.tensor.matmul(msg[0:D, :], l2s, Hs[:],
                                     start=True, stop=True)
                    # d-copy into the c-major etb strip (bias already in
                    # msg via the Hs ones row); DVE - ScalarE is full with
                    # both Prelu evacs
                    nc.vector.tensor_copy(etb[:, j * T:(j + 1) * T],
                                          msg[0:D, :])

                for j in range(GB):
                    r0 = F0[0:fr, j * T:(j + 1) * T]
                    r1 = F1[0:fr, j * T:(j + 1) * T]
                    Z0 = zp.tile([ZROWS, T], F32, tag="Z", name="Z")
                    nc.tensor.matmul(Z0[0:ZROWS, :], lb(i, 0, 0)[0:fr, :],
                                     r0, start=True, stop=False)
                    nc.tensor.matmul(Z0[0:ZROWS, :], lb(i, 0, 1)[0:fr, :],
                                     r0, start=False, stop=True)
                    Z1 = zp.tile([ZROWS, T], F32, tag="Z", name="Z")
                    nc.tensor.matmul(Z1[0:ZROWS, :], lb(i, 1, 0)[0:fr, :],
                                     r1, start=True, stop=False)
                    nc.tensor.matmul(Z1[0:ZROWS, :], lb(i, 1, 1)[0:fr, :],
                                     r1, start=False, stop=True)
                    H0 = hp.tile([ZROWS, T], F32, tag="H")
                    nc.scalar.activation(H0[:], Z0[0:ZROWS, :], AF.Prelu,
                                         bias=0.0, scale=1.0, alpha=SLOPE)
                    H1 = hp.tile([ZROWS, T], F32, tag="H")
                    nc.scalar.activation(H1[:], Z1[0:ZROWS, :], AF.Prelu,
                                         bias=0.0, scale=1.0, alpha=SLOPE)
                    Hs = hsp.tile([ZROWS, T], F32, tag="Hs")
                    nc.vector.tensor_add(Hs[:], H0[:], H1[:])
                    pend.append((j, Hs))
                    if len(pend) > 3:
                        mm2_stage(*pend.pop(0))
                    inject(j)
                while pend:
                    mm2_stage(*pend.pop(0))

            def emit_gn(Fs2, E, etb, i, b):
                """Emit gather+sq now; return dict j -> [callback] with the
                rest of the GN tail spread across the partner block's tile
                stream (each cross-engine hop gets ~2 tiles of slack)."""
                F0, F1 = Fs2
                if BENCH_SKIP_GN:
                    return {}
                # gather on the SWDGE (gpsimd) ring - HWDGE rings carry the
                # loads and refreshes
                stage = gp.tile([SROWS, T], F32, tag="stage", name="stage")
                nc.gpsimd.dma_start(out=stage[0:SROWS // 2, :],
                                    in_=etb[0:D // 2, :])
                nc.gpsimd.dma_start(out=stage[SROWS // 2:SROWS, :],
                                    in_=etb[D // 2:D, :])
                st = {}

                def p_sq():
                    st["sq"] = gp.tile([SROWS, T], F32, tag="sq", name="sq")
                    nc.vector.tensor_mul(st["sq"][:], stage[:], stage[:])

                def p_var():
                    st["var"] = sp.tile([SROWS, T], F32, tag="var",
                                        name="var")
                    nc.tensor.matmul(st["var"][0:SROWS, :], g1[:],
                                     st["sq"][:], start=True, stop=True)

                def p_rstd():
                    st["rstd"] = gp.tile([SROWS, T], F32, tag="rstd",
                                         name="rstd")
                    nc.scalar.activation(st["rstd"][:], st["var"][0:SROWS, :],
                                         AF.Abs_reciprocal_sqrt,
                                         bias=eps[0:SROWS, 0:1], scale=1.0)

                def p_y():
                    st["y"] = gp.tile([SROWS, T], F32, tag="y", name="y")
                    nc.vector.tensor_mul(st["y"][:], stage[:], st["rstd"][:])

                def p_upd():
                    st["upd"] = gp.tile([SROWS, T], F32, tag="upd",
                                        name="upd")
                    nc.scalar.activation(st["upd"][:], st["y"][:], AF.Prelu,
                                         bias=cg[:, 3 + i:4 + i],
                                         scale=cg[:, i:i + 1], alpha=SLOPE)

                def p_acc():
                    if i == 0:
                        nc.gpsimd.tensor_scalar(E[:], st["upd"][:], 1.0,
                                                None, OP.add)
                    else:
                        nc.gpsimd.tensor_add(E[:], E[:], st["upd"][:])

                def p_out():
                    if i < N_LAYERS - 1:
                        Ehi = gp.tile([SROWS, T], F16, tag="Ehi", name="Ehi")
                        nc.scalar.activation(Ehi[:], E[:], AF.Copy)
                        Elo = gp.tile([SROWS, T], F16, tag="Elo", name="Elo")
                        nc.vector.tensor_sub(Elo[:], E[:], Ehi[:])
                        # emb refresh into both halves (c-major flat match),
                        # spread across all three DMA rings
                        nc.sync.dma_start(out=F0[113:119, :], in_=Ehi[:])
                        nc.gpsimd.dma_start(out=F1[113:119, :], in_=Ehi[:])
                        nc.scalar.dma_start(out=F0[119:125, :], in_=Elo[:])
                        nc.gpsimd.dma_start(out=F1[119:125, :], in_=Elo[:])
                    else:
                        nc.gpsimd.dma_start(out=out_e[:, b * W:(b + 1) * W],
                                            in_=E[:])

                return {1: [p_var], 3: [p_rstd], 5: [p_y], 7: [p_upd],
                        9: [p_acc], 11: [p_out]}

            # Software-pipelined emission in even/odd pairs; each batch's
            # GN tail is deferred into the partner's tile stream.
            pairs = [(0, 1), (2, 3), (4, 5), (6, None)]
            Fs = {0: emit_load(0), 1: emit_load(1)}
            Es = {}
            tail = [None]

            def run_block(x, i, defer):
                etb = ep.tile([D, W], F32, tag="et", name="etb")
                emit_tiles(Fs[x], etb, i, after2=tail[0])
                tail[0] = None
                if i == N_LAYERS - 1 and x + 2 <= NB - 1:
                    Fs[x + 2] = emit_load(x + 2)
                fin = emit_gn(Fs[x], Es[x], etb, i, x)
                if defer:
                    tail[0] = fin
                else:
                    # lone block: its own next layer reads the refreshed
                    # emb rows, so the tail cannot be deferred
                    for j in sorted(fin):
                        for cb in fin[j]:
                            cb()

            for a, b in pairs:
                Es[a] = gp.tile([SROWS, T], F32, tag="E", name="E")
                if b is not None:
                    Es[b] = gp.tile([SROWS, T], F32, tag="E", name="E")
                for i in range(N_LAYERS):
                    for x in ((a, b) if b is not None else (a,)):
                        run_block(x, i, defer=(b is not None))

    print(f"[kernel] trace done {_time.time():.1f}", flush=True)
    nc.compile()
    print(f"[kernel] bacc compile done {_time.time():.1f}", flush=True)
    return nc


_RUNNER_CACHE = None


def get_runner():
    """Build (once) the jitted 8-core executable."""
    global _RUNNER_CACHE
    if _RUNNER_CACHE is not None:
        return _RUNNER_CACHE
    import jax
    import numpy as _np
    from jax.sharding import Mesh, PartitionSpec
    from jax.experimental.shard_map import shard_map
    import concourse.mybir as mybir
    from concourse.bass2jax import (
        install_neuronx_cc_hook, _bass_exec_p, partition_id_tensor)

    nc = _build_nc()
    install_neuronx_cc_hook()
    partition_name = nc.partition_id_tensor.name if nc.partition_id_tensor else None
    in_names, out_names, out_avals, zero_outs = [], [], [], []
    for alloc in nc.m.functions[0].allocations:
        if not isinstance(alloc, mybir.MemoryLocationSet):
            continue
        name = alloc.memorylocations[0].name
        if alloc.kind == "ExternalInput":
            if name != partition_name:
                in_names.append(name)
        elif alloc.kind == "ExternalOutput":
            out_names.append(name)
            shape = tuple(alloc.tensor_shape)
            dtype = mybir.dt.np(alloc.dtype)
            out_avals.append(jax.core.ShapedArray(shape, dtype))
            zero_outs.append(_np.zeros(shape, dtype))
    n_params = len(in_names)
    all_in_names = in_names + out_names
    if partition_name is not None:
        all_in_names.append(partition_name)

    def _body(*args):
        operands = list(args)
        if partition_name is not None:
            operands.append(partition_id_tensor())
        return tuple(_bass_exec_p.bind(
            *operands,
            out_avals=tuple(out_avals),
            in_names=tuple(all_in_names),
            out_names=tuple(out_names),
            lowering_input_output_aliases=(),
            sim_require_finite=True,
            sim_require_nnan=True,
            nc=nc,
        ))

    devices = jax.devices()[:N_CORES]
    mesh = Mesh(_np.asarray(devices), ("core",))
    nin = n_params + len(out_names)
    sharded = jax.jit(shard_map(
        _body, mesh=mesh,
        in_specs=(PartitionSpec("core"),) * nin,
        out_specs=(PartitionSpec("core"),) * len(out_names),
        check_rep=False))
    _RUNNER_CACHE = {
        "fn": sharded, "in_names": in_names, "out_names": out_names,
        "zero_outs": zero_outs, "mesh": mesh, "nc": nc,
    }
    return _RUNNER_CACHE


def run_cores(in_maps):
    """Run the 8-core kernel on a list of per-core input dicts."""
    import numpy as _np
    r = get_runner()
    concat_in = [
        _np.concatenate([in_maps[c][name] for c in range(N_CORES)], axis=0)
        for name in r["in_names"]
    ] + [_np.concatenate([z] * N_CORES, axis=0) for z in r["zero_outs"]]
    outs = r["fn"](*concat_in)
    res = []
    for c in range(N_CORES):
        d = {}
        for i, name in enumerate(r["out_names"]):
            full = _np.asarray(outs[i])
            rows = full.shape[0] // N_CORES
            d[name] = full[c * rows:(c + 1) * rows]
        res.append(d)
    return res


def make_in_maps(dist, atomtypes, W1, b1, W2, b2, gw, gb):
    dist = np.asarray(dist, np.float32)
    atomtypes = np.asarray(atomtypes, np.float32)
    B, N, _, _ = atomtypes.shape
    P = B * N
    assert P == N_CORES * PC

    l1, l2, G1, cstg = _pack_weights(
        np.asarray(W1, np.float32), np.asarray(b1, np.float32),
        np.asarray(W2, np.float32), np.asarray(b2, np.float32),
        np.asarray(gw, np.float32), np.asarray(gb, np.float32))

    atom = atomtypes.reshape(P, K, D)
    dst = dist.reshape(P, K)
    a_hi = np.float16(atom)
    a_lo = np.float16(atom - a_hi.astype(np.float32))
    d_hi = np.float16(dst)
    d_lo = np.float16(dst - d_hi.astype(np.float32))

    in_maps = []
    for c in range(N_CORES):
        sl = slice(c * PC, (c + 1) * PC)
        ads = []
        for h in range(2):
            ks = slice(h * 8, h * 8 + 8)
            a = np.zeros((PP, AD_ROWS), np.float16)
            a[:PC, 0:48] = a_hi[sl, ks].reshape(PC, 48)
            a[:PC, 48:96] = a_lo[sl, ks].reshape(PC, 48)
            a[:PC, 96:104] = d_hi[sl, ks]
            a[:PC, 104:112] = d_lo[sl, ks]
            a[:, 112] = np.float16(1.0)
            ads.append(np.ascontiguousarray(a.T))
        in_maps.append({
            "ad0": ads[0], "ad1": ads[1],
            "lhsT1": l1, "lhsT2": l2, "g1": G1, "cstg": cstg,
        })
    return in_maps, (B, N)


def kernel(dist, atomtypes, W1, b1, W2, b2, gw, gb):
    in_maps, (B, N) = make_in_maps(dist, atomtypes, W1, b1, W2, b2, gw, gb)
    res = run_cores(in_maps)
    outs = [res[c]["out"][:, :PC].T for c in range(N_CORES)]
    return np.concatenate(outs, axis=0).reshape(B, N, D).astype(np.float32)


if __name__ == "__main__":
    rng = np.random.default_rng(0)
    inputs = {
        "dist": rng.random((4, 100000, 16, 1), dtype=np.float32),
        "atomtypes": rng.random((4, 100000, 16, 6), dtype=np.float32),
        "W1": rng.random((3, 13, 13), dtype=np.float32) - 0.5,
        "b1": rng.random((3, 13), dtype=np.float32) - 0.5,
        "W2": rng.random((3, 13, 6), dtype=np.float32) - 0.5,
        "b2": rng.random((3, 6), dtype=np.float32) - 0.5,
        "gw": np.ones((3, 6), np.float32),
        "gb": np.zeros((3, 6), np.float32),
    }
    out = kernel(**inputs)
    print(out.shape, out.dtype)


# revision 28
# speedup vs baseline: 1.0532x; 1.0532x over previous
"""Trainium2 Bass kernel: 3-layer GNN message passing (atom embedding).

Data-parallel over the B*N=400000 point axis across 8 NeuronCores.

Numerics: layer 2's GroupNorm has near-degenerate groups (min var ~6e-7
<< eps=1e-5), so any absolute error upstream is amplified ~300x into the
output; plain bf16/fp16/fp32r matmuls all fail the 2e-2 gate (measured).
This kernel keeps fp32-grade accuracy while running the big matmul at
1 cycle/row (vs 4 for fp32) using exact fp16 hi/lo splits:

- Every input value x is split (on host) into hi = fp16(x) and
  lo = fp16(x - hi); fp16 subnormals are honored by the PE (measured:
  3-term hi/lo matmul reaches rel 2e-7), so no scaling is needed.
- mm1 per neighbor-half is TWO fp16 matmul passes into one PSUM tile:
    pass0: [Ahi; Alo] @ [Whi; Whi]  = A @ Whi   (A exact)
    pass1: [Ahi; Alo] @ [Wlo; 0  ]  = Ahi @ Wlo (correction)
  Dropped term Alo@Wlo ~ 2^-24. End-to-end rel err vs reference:
  1.5e-4 (simulated), 100x inside the gate.
- F tile per half: [125, W] fp16 = 48 atom_hi, 48 atom_lo, 8 dist_hi,
  8 dist_lo, 1 ones, 6 emb_hi, 6 emb_lo. Layer 0 folds emb (==1) into
  the ones row and reads only rows 0:113. lhsT pass0 col 104 puts 0.5
  on the ones row, so Prelu(z)[104] == 0.5 per half and the half-summed
  Hs row 104 == 1.0 - a free ones row for mm2's bias.
- mm2 is ONE fp32 matmul (4 cyc/row): lhsT [105, 6] = W2@C replicated
  over the 8 neighbor blocks + row 104 = C@(16 b2), where C is the
  GroupNorm centering matrix, so msg PSUM is already the centered d
  with bias included.
- Engine balance per tile: PE 4 fp16 passes + 1 fp32 (mm2, lag 2);
  ScalarE does both Prelu evacs (DVE cannot read 2 PSUM operands); DVE
  sums the halves (Hs = H0 + H1) and d-copies msg into the c-major etb
  strip; GPSIMD accumulates E.
- GroupNorm batched over GB=14 tiles in c-major [84, T] layout (gather
  DMA from etb on the SWDGE ring), stats matmul in fp32, emb refresh
  writes fp16 hi/lo pairs into both halves' F tiles.
- DMA-instruction parallelism is the scarce resource on this part
  (single big DMA ~34GB/s): each 1.6MB ad load is split into 8 row
  chunks alternating across both HWDGE rings; gather/store and half the
  refreshes ride the SWDGE (gpsimd) ring. This alone was worth ~25%.
- Batches run in software-pipelined even/odd pairs; each batch's GN
  tail is spread hop-by-hop (var/rstd/y/upd/E/refresh) across the
  partner block's tile stream so no engine queue ever blocks on a
  cross-engine chain.
"""
import os
import sys

sys.path.insert(0, "/opt/trn_rl_repo")

import numpy as np

# timing-bisect knobs (correctness is broken when set; bench only)
BENCH_SKIP_GN = bool(int(os.environ.get("BENCH_SKIP_GN", "0")))
BENCH_SKIP_MM = bool(int(os.environ.get("BENCH_SKIP_MM", "0")))

D = 6
K = 16
N_LAYERS = 3
C_IN = 13
EPS = 1e-5
SLOPE = 0.2

N_CORES = 8
T = 512            # points per tile (PSUM bank = 512 fp32)
GB = 14            # point-tiles per groupnorm batch
W = GB * T         # 7168 points per batch
NB = 7             # batches per core
PC = 50000         # points per core
PP = NB * W        # padded points per core = 50176
SROWS = 6 * GB     # 84

F_ROWS = 125       # 48+48+8+8+1 (hbm) + 6 emb_hi + 6 emb_lo (device)
AD_ROWS = 113      # rows loaded from HBM per half
ZROWS = 105        # mm1 out: 8 nbr x 13 + 1 ones


def _f16(x):
    return np.float16(x)


def _split16(x):
    hi = np.float16(x)
    lo = np.float16(x.astype(np.float32) - hi.astype(np.float32))
    return hi, lo


def _pack_weights(W1, b1, W2, b2, gw, gb):
    """Packed lhsT / const tensors (host side, a few KB)."""
    C = np.eye(D, dtype=np.float32) - np.kron(
        np.eye(2, dtype=np.float32), np.ones((3, 3), np.float32) / 3.0)

    # mm1 lhsT: [125, 3 layers * 2 halves * 2 passes * 105] fp16
    l1 = np.zeros((N_LAYERS, 2, 2, F_ROWS, ZROWS), np.float16)
    for i in range(N_LAYERS):
        Wa = W1[i, 6:12, :]        # atom rows [6,13]
        Wd = W1[i, 12:13, :]       # dist row  [1,13]
        We = W1[i, 0:6, :]         # emb rows  [6,13]
        bias = b1[i] + (We.sum(axis=0) if i == 0 else 0.0)
        Wa_h, Wa_l = _split16(Wa)
        Wd_h, Wd_l = _split16(Wd)
        We_h, We_l = _split16(We)
        b_h, b_l = _split16(bias)
        for h in range(2):
            for k8 in range(8):
                cols = slice(k8 * 13, k8 * 13 + 13)
                # pass0: A @ Whi  (hi AND lo rows carry Whi)
                L = l1[i, h, 0]
                L[k8 * 6:(k8 + 1) * 6, cols] = Wa_h
                L[48 + k8 * 6:48 + (k8 + 1) * 6, cols] = Wa_h
                L[96 + k8, cols] = Wd_h
                L[104 + k8, cols] = Wd_h
                L[112, cols] = b_h
                if i > 0:
                    L[113:119, cols] = We_h
                    L[119:125, cols] = We_h
                # pass1: Ahi @ Wlo  (hi rows only)
                L = l1[i, h, 1]
                L[k8 * 6:(k8 + 1) * 6, cols] = Wa_l
                L[96 + k8, cols] = Wd_l
                L[112, cols] = b_l
                if i > 0:
                    L[113:119, cols] = We_l
            # ones output column for mm2 bias: Prelu(0.5)+Prelu(0.5)=1
            l1[i, h, 0][112, 104] = np.float16(0.5)
    l1_flat = np.ascontiguousarray(
        np.concatenate([l1[i, h, p] for i in range(N_LAYERS)
                        for h in range(2) for p in range(2)], axis=1))

    # mm2 lhsT: [105, 18] fp32; row 104 = centered bias
    l2 = np.zeros((ZROWS, N_LAYERS * D), np.float32)
    for i in range(N_LAYERS):
        W2C = (W2[i] @ C).astype(np.float32)
        for k8 in range(8):
            l2[k8 * 13:k8 * 13 + 13, i * D:(i + 1) * D] = W2C
        l2[104, i * D:(i + 1) * D] = C @ (16.0 * b2[i])

    # c-major GroupNorm averaging matrix: p = c*GB + j
    G1 = np.zeros((SROWS, SROWS), np.float32)
    for p in range(SROWS):
        for q in range(SROWS):
            if p % GB == q % GB and (p // GB) // 3 == (q // GB) // 3:
                G1[p, q] = 1.0 / 3.0

    cstg = np.zeros((SROWS, 2 * N_LAYERS), np.float32)
    for p in range(SROWS):
        c = p // GB
        for i in range(N_LAYERS):
            cstg[p, i] = gw[i, c]
            cstg[p, 3 + i] = gb[i, c]
    return l1_flat, l2, G1, cstg


def _build_nc():
    import concourse.bass as bass
    import concourse.bacc as bacc
    import concourse.mybir as mybir
    from concourse import tile

    F32 = mybir.dt.float32
    F16 = mybir.dt.float16
    AF = mybir.ActivationFunctionType
    OP = mybir.AluOpType

    import time as _time
    print(f"[kernel] build start {_time.time():.1f}", flush=True)
    nc = bacc.Bacc("TRN2", target_bir_lowering=False)
    ad0_e = nc.declare_dram_parameter("ad0", [AD_ROWS, PP], F16, isOutput=False)
    ad1_e = nc.declare_dram_parameter("ad1", [AD_ROWS, PP], F16, isOutput=False)
    l1_e = nc.declare_dram_parameter("lhsT1", [F_ROWS, 12 * ZROWS], F16,
                                     isOutput=False)
    l2_e = nc.declare_dram_parameter("lhsT2", [ZROWS, N_LAYERS * D], F32,
                                     isOutput=False)
    g1_e = nc.declare_dram_parameter("g1", [SROWS, SROWS], F32, isOutput=False)
    cg_e = nc.declare_dram_parameter("cstg", [SROWS, 2 * N_LAYERS], F32,
                                     isOutput=False)
    out_e = nc.declare_dram_parameter("out", [D, PP], F32, isOutput=True)

    with tile.TileContext(nc) as tc:
        with tc.tile_pool(name="w", bufs=1) as wp, \
             tc.tile_pool(name="f0", bufs=2) as fp0, \
             tc.tile_pool(name="f1", bufs=2) as fp1, \
             tc.tile_pool(name="e", bufs=2) as ep, \
             tc.tile_pool(name="h", bufs=4) as hp, \
             tc.tile_pool(name="hs", bufs=4) as hsp, \
             tc.tile_pool(name="g", bufs=3) as gp, \
             tc.tile_pool(name="z", bufs=5, space="PSUM") as zp, \
             tc.tile_pool(name="m", bufs=2, space="PSUM") as mp, \
             tc.tile_pool(name="s", bufs=1, space="PSUM") as sp:
            l1 = wp.tile([F_ROWS, 12 * ZROWS], F16)
            l2 = wp.tile([ZROWS, N_LAYERS * D], F32)
            g1 = wp.tile([SROWS, SROWS], F32)
            cg = wp.tile([SROWS, 2 * N_LAYERS], F32)
            eps = wp.tile([128, 1], F32)
            dmy = wp.tile([D, T], F32)
            nc.gpsimd.memset(dmy[:], 0.01)
            nc.sync.dma_start(out=l1[:], in_=l1_e[:])
            nc.sync.dma_start(out=l2[:], in_=l2_e[:])
            nc.sync.dma_start(out=g1[:], in_=g1_e[:])
            nc.sync.dma_start(out=cg[:], in_=cg_e[:])
            nc.gpsimd.memset(eps[:], EPS)

            def lb(i, h, p):
                off = ((i * 2 + h) * 2 + p) * ZROWS
                return l1[:, off:off + ZROWS]

            def emit_load(b):
                # each load split into row chunks across both HWDGE rings:
                # more DMA instructions in flight -> more engine parallelism
                F0 = fp0.tile([F_ROWS, W], F16, tag="F0", name="F0")
                F1 = fp1.tile([F_ROWS, W], F16, tag="F1", name="F1")
                cuts = [0, 15, 29, 43, 57, 71, 85, 99, AD_ROWS]
                for k in range(8):
                    r0, r1 = cuts[k], cuts[k + 1]
                    if k == 7:
                        # one chunk per half rides the idle SWDGE ring
                        nc.gpsimd.dma_start(
                            out=F0[r0:r1, :],
                            in_=ad0_e[r0:r1, b * W:(b + 1) * W])
                        nc.gpsimd.dma_start(
                            out=F1[r0:r1, :],
                            in_=ad1_e[r0:r1, b * W:(b + 1) * W])
                        continue
                    ring = nc.sync if k % 2 == 0 else nc.scalar
                    ring.dma_start(out=F0[r0:r1, :],
                                   in_=ad0_e[r0:r1, b * W:(b + 1) * W])
                    ring2 = nc.scalar if k % 2 == 0 else nc.sync
                    ring2.dma_start(out=F1[r0:r1, :],
                                    in_=ad1_e[r0:r1, b * W:(b + 1) * W])
                return (F0, F1)

            def emit_tiles(Fs2, etb, i, after2=None):
                """after2: dict j -> list of callbacks injected after tile
                j's emission (spread GN-tail hops across the stream)."""
                F0, F1 = Fs2
                fr = AD_ROWS if (i == 0 or BENCH_SKIP_GN) else F_ROWS
                l2s = l2[:, i * D:(i + 1) * D]
                after2 = after2 or {}
                pend = []

                def inject(j):
                    for cb in after2.get(j, ()):
                        cb()

                if BENCH_SKIP_MM:
                    for j in range(GB):
                        nc.vector.tensor_copy(etb[:, j * T:(j + 1) * T],
                                              dmy[:])
                        inject(j)
                    return

                def mm2_stage(j, Hs):
                    msg = mp.tile([D, T], F32, tag="msg", name="msg")
                    nc.tensor.matmul(msg[0:D, :], l2s, Hs[:],
                                     start=True, stop=True)
                    # d-copy into the c-major etb strip (bias already in
                    # msg via the Hs ones row); DVE - ScalarE is full with
                    # both Prelu evacs
                    nc.vector.tensor_copy(etb[:, j * T:(j + 1) * T],
                                          msg[0:D, :])

                for j in range(GB):
                    r0 = F0[0:fr, j * T:(j + 1) * T]
                    r1 = F1[0:fr, j * T:(j + 1) * T]
                    Z0 = zp.tile([ZROWS, T], F32, tag="Z", name="Z")
                    nc.tensor.matmul(Z0[0:ZROWS, :], lb(i, 0, 0)[0:fr, :],
                                     r0, start=True, stop=False)
                    nc.tensor.matmul(Z0[0:ZROWS, :], lb(i, 0, 1)[0:fr, :],
                                     r0, start=False, stop=True)
                    Z1 = zp.tile([ZROWS, T], F32, tag="Z", name="Z")
                    nc.tensor.matmul(Z1[0:ZROWS, :], lb(i, 1, 0)[0:fr, :],
                                     r1, start=True, stop=False)
                    nc.tensor.matmul(Z1[0:ZROWS, :], lb(i, 1, 1)[0:fr, :],
                                     r1, start=False, stop=True)
                    H0 = hp.tile([ZROWS, T], F32, tag="H")
                    nc.scalar.activation(H0[:], Z0[0:ZROWS, :], AF.Prelu,
                                         bias=0.0, scale=1.0, alpha=SLOPE)
                    H1 = hp.tile([ZROWS, T], F32, tag="H")
                    nc.scalar.activation(H1[:], Z1[0:ZROWS, :], AF.Prelu,
                                         bias=0.0, scale=1.0, alpha=SLOPE)
                    Hs = hsp.tile([ZROWS, T], F32, tag="Hs")
                    nc.vector.tensor_add(Hs[:], H0[:], H1[:])
                    pend.append((j, Hs))
                    if len(pend) > 3:
                        mm2_stage(*pend.pop(0))
                    inject(j)
                while pend:
                    mm2_stage(*pend.pop(0))

            def emit_gn(Fs2, E, etb, i, b):
                """Emit gather+sq now; return dict j -> [callback] with the
                rest of the GN tail spread across the partner block's tile
                stream (each cross-engine hop gets ~2 tiles of slack)."""
                F0, F1 = Fs2
                if BENCH_SKIP_GN:
                    return {}
                # gather on the SWDGE (gpsimd) ring - HWDGE rings carry the
                # loads and refreshes
                stage = gp.tile([SROWS, T], F32, tag="stage", name="stage")
                nc.gpsimd.dma_start(out=stage[0:SROWS // 2, :],
                                    in_=etb[0:D // 2, :])
                nc.gpsimd.dma_start(out=stage[SROWS // 2:SROWS, :],
                                    in_=etb[D // 2:D, :])
                st = {}

                def p_sq():
                    st["sq"] = gp.tile([SROWS, T], F32, tag="sq", name="sq")
                    nc.vector.tensor_mul(st["sq"][:], stage[:], stage[:])

                def p_var():
                    st["var"] = sp.tile([SROWS, T], F32, tag="var",
                                        name="var")
                    nc.tensor.matmul(st["var"][0:SROWS, :], g1[:],
                                     st["sq"][:], start=True, stop=True)

                def p_rstd():
                    st["rstd"] = gp.tile([SROWS, T], F32, tag="rstd",
                                         name="rstd")
                    nc.scalar.activation(st["rstd"][:], st["var"][0:SROWS, :],
                                         AF.Abs_reciprocal_sqrt,
                                         bias=eps[0:SROWS, 0:1], scale=1.0)

                def p_y():
                    st["y"] = gp.tile([SROWS, T], F32, tag="y", name="y")
                    nc.vector.tensor_mul(st["y"][:], stage[:], st["rstd"][:])

                def p_upd():
                    st["upd"] = gp.tile([SROWS, T], F32, tag="upd",
                                        name="upd")
                    nc.scalar.activation(st["upd"][:], st["y"][:], AF.Prelu,
                                         bias=cg[:, 3 + i:4 + i],
                                         scale=cg[:, i:i + 1], alpha=SLOPE)

                def p_acc():
                    if i == 0:
                        nc.gpsimd.tensor_scalar(E[:], st["upd"][:], 1.0,
                                                None, OP.add)
                    else:
                        nc.gpsimd.tensor_add(E[:], E[:], st["upd"][:])

                def p_out():
                    if i < N_LAYERS - 1:
                        Ehi = gp.tile([SROWS, T], F16, tag="Ehi", name="Ehi")
                        nc.scalar.activation(Ehi[:], E[:], AF.Copy)
                        Elo = gp.tile([SROWS, T], F16, tag="Elo", name="Elo")
                        nc.vector.tensor_sub(Elo[:], E[:], Ehi[:])
                        # emb refresh into both halves (c-major flat match),
                        # spread across all three DMA rings
                        nc.sync.dma_start(out=F0[113:119, :], in_=Ehi[:])
                        nc.gpsimd.dma_start(out=F1[113:119, :], in_=Ehi[:])
                        nc.scalar.dma_start(out=F0[119:125, :], in_=Elo[:])
                        nc.gpsimd.dma_start(out=F1[119:125, :], in_=Elo[:])
                    else:
                        nc.gpsimd.dma_start(out=out_e[:, b * W:(b + 1) * W],
                                            in_=E[:])

                return {0: [p_sq], 2: [p_var], 4: [p_rstd], 6: [p_y],
                        8: [p_upd], 10: [p_acc], 12: [p_out]}

            # Software-pipelined emission in even/odd pairs; each batch's
            # GN tail is deferred into the partner's tile stream.
            pairs = [(0, 1), (2, 3), (4, 5), (6, None)]
            Fs = {0: emit_load(0), 1: emit_load(1)}
            Es = {}
            tail = [None]

            def run_block(x, i, defer):
                etb = ep.tile([D, W], F32, tag="et", name="etb")
                emit_tiles(Fs[x], etb, i, after2=tail[0])
                tail[0] = None
                if i == N_LAYERS - 1 and x + 2 <= NB - 1:
                    Fs[x + 2] = emit_load(x + 2)
                fin = emit_gn(Fs[x], Es[x], etb, i, x)
                if defer:
                    tail[0] = fin
                else:
                    # lone block: its own next layer reads the refreshed
                    # emb rows, so the tail cannot be deferred
                    for j in sorted(fin):
                        for cb in fin[j]:
                            cb()

            for a, b in pairs:
                Es[a] = gp.tile([SROWS, T], F32, tag="E", name="E")
                if b is not None:
                    Es[b] = gp.tile([SROWS, T], F32, tag="E", name="E")
                for i in range(N_LAYERS):
                    for x in ((a, b) if b is not None else (a,)):
                        run_block(x, i, defer=(b is not None))

    print(f"[kernel] trace done {_time.time():.1f}", flush=True)
    nc.compile()
    print(f"[kernel] bacc compile done {_time.time():.1f}", flush=True)
    return nc


_RUNNER_CACHE = None


def get_runner():
    """Build (once) the jitted 8-core executable."""
    global _RUNNER_CACHE
    if _RUNNER_CACHE is not None:
        return _RUNNER_CACHE
    import jax
    import numpy as _np
    from jax.sharding import Mesh, PartitionSpec
    from jax.experimental.shard_map import shard_map
    import concourse.mybir as mybir
    from concourse.bass2jax import (
        install_neuronx_cc_hook, _bass_exec_p, partition_id_tensor)

    nc = _build_nc()
    install_neuronx_cc_hook()
    partition_name = nc.partition_id_tensor.name if nc.partition_id_tensor else None
    in_names, out_names, out_avals, zero_outs = [], [], [], []
    for alloc in nc.m.functions[0].allocations:
        if not isinstance(alloc, mybir.MemoryLocationSet):
            continue
        name = alloc.memorylocations[0].name
        if alloc.kind == "ExternalInput":
            if name != partition_name:
                in_names.append(name)
        elif alloc.kind == "ExternalOutput":
            out_names.append(name)
            shape = tuple(alloc.tensor_shape)
            dtype = mybir.dt.np(alloc.dtype)
            out_avals.append(jax.core.ShapedArray(shape, dtype))
            zero_outs.append(_np.zeros(shape, dtype))
    n_params = len(in_names)
    all_in_names = in_names + out_names
    if partition_name is not None:
        all_in_names.append(partition_name)

    def _body(*args):
        operands = list(args)
        if partition_name is not None:
            operands.append(partition_id_tensor())
        return tuple(_bass_exec_p.bind(
            *operands,
            out_avals=tuple(out_avals),
            in_names=tuple(all_in_names),
            out_names=tuple(out_names),
            lowering_input_output_aliases=(),
            sim_require_finite=True,
            sim_require_nnan=True,
            nc=nc,
        ))

    devices = jax.devices()[:N_CORES]
    mesh = Mesh(_np.asarray(devices), ("core",))
    nin = n_params + len(out_names)
    sharded = jax.jit(shard_map(
        _body, mesh=mesh,
        in_specs=(PartitionSpec("core"),) * nin,
        out_specs=(PartitionSpec("core"),) * len(out_names),
        check_rep=False))
    _RUNNER_CACHE = {
        "fn": sharded, "in_names": in_names, "out_names": out_names,
        "zero_outs": zero_outs, "mesh": mesh, "nc": nc,
    }
    return _RUNNER_CACHE


def run_cores(in_maps):
    """Run the 8-core kernel on a list of per-core input dicts."""
    import numpy as _np
    r = get_runner()
    concat_in = [
        _np.concatenate([in_maps[c][name] for c in range(N_CORES)], axis=0)
        for name in r["in_names"]
    ] + [_np.concatenate([z] * N_CORES, axis=0) for z in r["zero_outs"]]
    outs = r["fn"](*concat_in)
    res = []
    for c in range(N_CORES):
        d = {}
        for i, name in enumerate(r["out_names"]):
            full = _np.asarray(outs[i])
            rows = full.shape[0] // N_CORES
            d[name] = full[c * rows:(c + 1) * rows]
        res.append(d)
    return res


def make_in_maps(dist, atomtypes, W1, b1, W2, b2, gw, gb):
    dist = np.asarray(dist, np.float32)
    atomtypes = np.asarray(atomtypes, np.float32)
    B, N, _, _ = atomtypes.shape
    P = B * N
    assert P == N_CORES * PC

    l1, l2, G1, cstg = _pack_weights(
        np.asarray(W1, np.float32), np.asarray(b1, np.float32),
        np.asarray(W2, np.float32), np.asarray(b2, np.float32),
        np.asarray(gw, np.float32), np.asarray(gb, np.float32))

    atom = atomtypes.reshape(P, K, D)
    dst = dist.reshape(P, K)
    a_hi = np.float16(atom)
    a_lo = np.float16(atom - a_hi.astype(np.float32))
    d_hi = np.float16(dst)
    d_lo = np.float16(dst - d_hi.astype(np.float32))

    in_maps = []
    for c in range(N_CORES):
        sl = slice(c * PC, (c + 1) * PC)
        ads = []
        for h in range(2):
            ks = slice(h * 8, h * 8 + 8)
            a = np.zeros((PP, AD_ROWS), np.float16)
            a[:PC, 0:48] = a_hi[sl, ks].reshape(PC, 48)
            a[:PC, 48:96] = a_lo[sl, ks].reshape(PC, 48)
            a[:PC, 96:104] = d_hi[sl, ks]
            a[:PC, 104:112] = d_lo[sl, ks]
            a[:, 112] = np.float16(1.0)
            ads.append(np.ascontiguousarray(a.T))
        in_maps.append({
            "ad0": ads[0], "ad1": ads[1],
            "lhsT1": l1, "lhsT2": l2, "g1": G1, "cstg": cstg,
        })
    return in_maps, (B, N)


def kernel(dist, atomtypes, W1, b1, W2, b2, gw, gb):
    in_maps, (B, N) = make_in_maps(dist, atomtypes, W1, b1, W2, b2, gw, gb)
    res = run_cores(in_maps)
    outs = [res[c]["out"][:, :PC].T for c in range(N_CORES)]
    return np.concatenate(outs, axis=0).reshape(B, N, D).astype(np.float32)


if __name__ == "__main__":
    rng = np.random.default_rng(0)
    inputs = {
        "dist": rng.random((4, 100000, 16, 1), dtype=np.float32),
        "atomtypes": rng.random((4, 100000, 16, 6), dtype=np.float32),
        "W1": rng.random((3, 13, 13), dtype=np.float32) - 0.5,
        "b1": rng.random((3, 13), dtype=np.float32) - 0.5,
        "W2": rng.random((3, 13, 6), dtype=np.float32) - 0.5,
        "b2": rng.random((3, 6), dtype=np.float32) - 0.5,
        "gw": np.ones((3, 6), np.float32),
        "gb": np.zeros((3, 6), np.float32),
    }
    out = kernel(**inputs)
    print(out.shape, out.dtype)


# revision 30
# speedup vs baseline: 1.3256x; 1.2586x over previous
"""Trainium2 Bass kernel: 3-layer GNN message passing (atom embedding).

Data-parallel over the B*N=400000 point axis across 8 NeuronCores.

Numerics: layer 2's GroupNorm has near-degenerate groups (min var ~6e-7
<< eps=1e-5), so any absolute error upstream is amplified ~300x into the
output; plain bf16/fp16/fp32r matmuls all fail the 2e-2 gate (measured).
This kernel keeps fp32-grade accuracy while running the big matmul at
1 cycle/row (vs 4 for fp32) using exact fp16 hi/lo splits:

- Every input value x is split (on host) into hi = fp16(x) and
  lo = fp16(x - hi); fp16 subnormals are honored by the PE (measured:
  3-term hi/lo matmul reaches rel 2e-7), so no scaling is needed.
- mm1 per neighbor-half is TWO fp16 matmul passes into one PSUM tile:
    pass0: [Ahi; Alo] @ [Whi; Whi]  = A @ Whi   (A exact)
    pass1: [Ahi; Alo] @ [Wlo; 0  ]  = Ahi @ Wlo (correction)
  Dropped term Alo@Wlo ~ 2^-24. End-to-end rel err vs reference:
  1.5e-4 (simulated), 100x inside the gate.
- F tile per half: [125, W] fp16 = 48 atom_hi, 48 atom_lo, 8 dist_hi,
  8 dist_lo, 1 ones, 6 emb_hi, 6 emb_lo. Layer 0 folds emb (==1) into
  the ones row and reads only rows 0:113. lhsT pass0 col 104 puts 0.5
  on the ones row, so Prelu(z)[104] == 0.5 per half and the half-summed
  Hs row 104 == 1.0 - a free ones row for mm2's bias.
- mm2 is ONE fp32 matmul (4 cyc/row): lhsT [105, 6] = W2@C replicated
  over the 8 neighbor blocks + row 104 = C@(16 b2), where C is the
  GroupNorm centering matrix, so msg PSUM is already the centered d
  with bias included.
- Engine balance per tile: PE 4 fp16 passes + 1 fp32 (mm2, lag 2);
  ScalarE does both Prelu evacs (DVE cannot read 2 PSUM operands); DVE
  sums the halves (Hs = H0 + H1) and d-copies msg into the c-major etb
  strip; GPSIMD accumulates E.
- GroupNorm batched over GB=14 tiles in c-major [84, T] layout (gather
  DMA from etb on the SWDGE ring), stats matmul in fp32, emb refresh
  writes fp16 hi/lo pairs into both halves' F tiles.
- DMA-instruction parallelism is the scarce resource on this part
  (single big DMA ~34GB/s): each 1.6MB ad load is split into 8 row
  chunks alternating across both HWDGE rings; gather/store and half the
  refreshes ride the SWDGE (gpsimd) ring. This alone was worth ~25%.
- Batches run in software-pipelined even/odd pairs; each batch's GN
  tail is spread hop-by-hop (var/rstd/y/upd/E/refresh) across the
  partner block's tile stream so no engine queue ever blocks on a
  cross-engine chain.
"""
import os
import sys

sys.path.insert(0, "/opt/trn_rl_repo")

import numpy as np

# timing-bisect knobs (correctness is broken when set; bench only)
BENCH_SKIP_GN = bool(int(os.environ.get("BENCH_SKIP_GN", "0")))
BENCH_SKIP_MM = bool(int(os.environ.get("BENCH_SKIP_MM", "0")))

D = 6
K = 16
N_LAYERS = 3
C_IN = 13
EPS = 1e-5
SLOPE = 0.2

N_CORES = 8
T = 512            # points per tile (PSUM bank = 512 fp32)
GB = 14            # point-tiles per groupnorm batch
W = GB * T         # 7168 points per batch
NB = 7             # batches per core
PC = 50000         # points per core
PP = NB * W        # padded points per core = 50176
SROWS = 6 * GB     # 84

F_ROWS = 125       # 48+48+8+8+1 (hbm) + 6 emb_hi + 6 emb_lo (device)
AD_ROWS = 113      # rows loaded from HBM per half
ZROWS = 105        # mm1 out: 8 nbr x 13 + 1 ones


def _f16(x):
    return np.float16(x)


def _split16(x):
    hi = np.float16(x)
    lo = np.float16(x.astype(np.float32) - hi.astype(np.float32))
    return hi, lo


def _pack_weights(W1, b1, W2, b2, gw, gb):
    """Packed lhsT / const tensors (host side, a few KB)."""
    C = np.eye(D, dtype=np.float32) - np.kron(
        np.eye(2, dtype=np.float32), np.ones((3, 3), np.float32) / 3.0)

    # mm1 lhsT: [125, 3 layers * 2 halves * 2 passes * 105] fp16
    l1 = np.zeros((N_LAYERS, 2, 2, F_ROWS, ZROWS), np.float16)
    for i in range(N_LAYERS):
        Wa = W1[i, 6:12, :]        # atom rows [6,13]
        Wd = W1[i, 12:13, :]       # dist row  [1,13]
        We = W1[i, 0:6, :]         # emb rows  [6,13]
        bias = b1[i] + (We.sum(axis=0) if i == 0 else 0.0)
        Wa_h, Wa_l = _split16(Wa)
        Wd_h, Wd_l = _split16(Wd)
        We_h, We_l = _split16(We)
        b_h, b_l = _split16(bias)
        for h in range(2):
            for k8 in range(8):
                cols = slice(k8 * 13, k8 * 13 + 13)
                # pass0: A @ Whi  (hi AND lo rows carry Whi)
                L = l1[i, h, 0]
                L[k8 * 6:(k8 + 1) * 6, cols] = Wa_h
                L[48 + k8 * 6:48 + (k8 + 1) * 6, cols] = Wa_h
                L[96 + k8, cols] = Wd_h
                L[104 + k8, cols] = Wd_h
                L[112, cols] = b_h
                if i > 0:
                    L[113:119, cols] = We_h
                    L[119:125, cols] = We_h
                # pass1: Ahi @ Wlo  (hi rows only)
                L = l1[i, h, 1]
                L[k8 * 6:(k8 + 1) * 6, cols] = Wa_l
                L[96 + k8, cols] = Wd_l
                L[112, cols] = b_l
                if i > 0:
                    L[113:119, cols] = We_l
            # ones output column for mm2 bias: Prelu(0.5)+Prelu(0.5)=1
            l1[i, h, 0][112, 104] = np.float16(0.5)
    l1_flat = np.ascontiguousarray(
        np.concatenate([l1[i, h, p] for i in range(N_LAYERS)
                        for h in range(2) for p in range(2)], axis=1))

    # mm2 lhsT: [105, 18] fp32; row 104 = centered bias
    l2 = np.zeros((ZROWS, N_LAYERS * D), np.float32)
    for i in range(N_LAYERS):
        W2C = (W2[i] @ C).astype(np.float32)
        for k8 in range(8):
            l2[k8 * 13:k8 * 13 + 13, i * D:(i + 1) * D] = W2C
        l2[104, i * D:(i + 1) * D] = C @ (16.0 * b2[i])

    # c-major GroupNorm averaging matrix: p = c*GB + j
    G1 = np.zeros((SROWS, SROWS), np.float32)
    for p in range(SROWS):
        for q in range(SROWS):
            if p % GB == q % GB and (p // GB) // 3 == (q // GB) // 3:
                G1[p, q] = 1.0 / 3.0

    cstg = np.zeros((SROWS, 2 * N_LAYERS), np.float32)
    for p in range(SROWS):
        c = p // GB
        for i in range(N_LAYERS):
            cstg[p, i] = gw[i, c]
            cstg[p, 3 + i] = gb[i, c]
    return l1_flat, l2, G1, cstg


def _build_nc():
    import concourse.bass as bass
    import concourse.bacc as bacc
    import concourse.mybir as mybir
    from concourse import tile

    F32 = mybir.dt.float32
    F16 = mybir.dt.float16
    AF = mybir.ActivationFunctionType
    OP = mybir.AluOpType

    import time as _time
    print(f"[kernel] build start {_time.time():.1f}", flush=True)
    nc = bacc.Bacc("TRN2", target_bir_lowering=False)
    ad0_e = nc.declare_dram_parameter("ad0", [AD_ROWS, PP], F16, isOutput=False)
    ad1_e = nc.declare_dram_parameter("ad1", [AD_ROWS, PP], F16, isOutput=False)
    l1_e = nc.declare_dram_parameter("lhsT1", [F_ROWS, 12 * ZROWS], F16,
                                     isOutput=False)
    l2_e = nc.declare_dram_parameter("lhsT2", [ZROWS, N_LAYERS * D], F32,
                                     isOutput=False)
    g1_e = nc.declare_dram_parameter("g1", [SROWS, SROWS], F32, isOutput=False)
    cg_e = nc.declare_dram_parameter("cstg", [SROWS, 2 * N_LAYERS], F32,
                                     isOutput=False)
    out_e = nc.declare_dram_parameter("out", [D, PP], F32, isOutput=True)

    with tile.TileContext(nc) as tc:
        with tc.tile_pool(name="w", bufs=1) as wp, \
             tc.tile_pool(name="f0", bufs=2) as fp0, \
             tc.tile_pool(name="f1", bufs=2) as fp1, \
             tc.tile_pool(name="e", bufs=2) as ep, \
             tc.tile_pool(name="h", bufs=4) as hp, \
             tc.tile_pool(name="hs", bufs=4) as hsp, \
             tc.tile_pool(name="g", bufs=3) as gp, \
             tc.tile_pool(name="z", bufs=5, space="PSUM") as zp, \
             tc.tile_pool(name="m", bufs=2, space="PSUM") as mp, \
             tc.tile_pool(name="s", bufs=1, space="PSUM") as sp:
            l1 = wp.tile([F_ROWS, 12 * ZROWS], F16)
            l2 = wp.tile([ZROWS, N_LAYERS * D], F32)
            g1 = wp.tile([SROWS, SROWS], F32)
            cg = wp.tile([SROWS, 2 * N_LAYERS], F32)
            eps = wp.tile([128, 1], F32)
            dmy = wp.tile([D, T], F32)
            nc.gpsimd.memset(dmy[:], 0.01)
            nc.sync.dma_start(out=l1[:], in_=l1_e[:])
            nc.sync.dma_start(out=l2[:], in_=l2_e[:])
            nc.sync.dma_start(out=g1[:], in_=g1_e[:])
            nc.sync.dma_start(out=cg[:], in_=cg_e[:])
            nc.gpsimd.memset(eps[:], EPS)

            def lb(i, h, p):
                off = ((i * 2 + h) * 2 + p) * ZROWS
                return l1[:, off:off + ZROWS]

            def emit_load(b):
                # each load split into row chunks across both HWDGE rings:
                # more DMA instructions in flight -> more engine parallelism
                F0 = fp0.tile([F_ROWS, W], F16, tag="F0", name="F0")
                F1 = fp1.tile([F_ROWS, W], F16, tag="F1", name="F1")
                cuts = [0, 15, 29, 43, 57, 71, 85, 99, AD_ROWS]
                for k in range(8):
                    r0, r1 = cuts[k], cuts[k + 1]
                    if k == 7:
                        # one chunk per half rides the idle SWDGE ring
                        nc.gpsimd.dma_start(
                            out=F0[r0:r1, :],
                            in_=ad0_e[r0:r1, b * W:(b + 1) * W])
                        nc.gpsimd.dma_start(
                            out=F1[r0:r1, :],
                            in_=ad1_e[r0:r1, b * W:(b + 1) * W])
                        continue
                    ring = nc.sync if k % 2 == 0 else nc.scalar
                    ring.dma_start(out=F0[r0:r1, :],
                                   in_=ad0_e[r0:r1, b * W:(b + 1) * W])
                    ring2 = nc.scalar if k % 2 == 0 else nc.sync
                    ring2.dma_start(out=F1[r0:r1, :],
                                    in_=ad1_e[r0:r1, b * W:(b + 1) * W])
                return (F0, F1)

            def emit_tiles(Fs2, etb, i, after2=None):
                """after2: dict j -> list of callbacks injected after tile
                j's emission (spread GN-tail hops across the stream)."""
                F0, F1 = Fs2
                fr = AD_ROWS if (i == 0 or BENCH_SKIP_GN) else F_ROWS
                l2s = l2[:, i * D:(i + 1) * D]
                after2 = after2 or {}
                pend = []

                def inject(j):
                    for cb in after2.get(j, ()):
                        cb()

                if BENCH_SKIP_MM:
                    for j in range(GB):
                        nc.vector.tensor_copy(etb[:, j * T:(j + 1) * T],
                                              dmy[:])
                        inject(j)
                    return

                def mm2_stage(j, Hs):
                    msg = mp.tile([D, T], F32, tag="msg", name="msg")
                    nc.tensor.matmul(msg[0:D, :], l2s, Hs[:],
                                     start=True, stop=True)
                    # d-copy into the c-major etb strip (bias already in
                    # msg via the Hs ones row); DVE - ScalarE is full with
                    # both Prelu evacs
                    nc.vector.tensor_copy(etb[:, j * T:(j + 1) * T],
                                          msg[0:D, :])

                for j in range(GB):
                    r0 = F0[0:fr, j * T:(j + 1) * T]
                    r1 = F1[0:fr, j * T:(j + 1) * T]
                    Z0 = zp.tile([ZROWS, T], F32, tag="Z", name="Z")
                    nc.tensor.matmul(Z0[0:ZROWS, :], lb(i, 0, 0)[0:fr, :],
                                     r0, start=True, stop=False)
                    nc.tensor.matmul(Z0[0:ZROWS, :], lb(i, 0, 1)[0:fr, :],
                                     r0, start=False, stop=True)
                    Z1 = zp.tile([ZROWS, T], F32, tag="Z", name="Z")
                    nc.tensor.matmul(Z1[0:ZROWS, :], lb(i, 1, 0)[0:fr, :],
                                     r1, start=True, stop=False)
                    nc.tensor.matmul(Z1[0:ZROWS, :], lb(i, 1, 1)[0:fr, :],
                                     r1, start=False, stop=True)
                    H0 = hp.tile([ZROWS, T], F32, tag="H")
                    nc.scalar.activation(H0[:], Z0[0:ZROWS, :], AF.Prelu,
                                         bias=0.0, scale=1.0, alpha=SLOPE)
                    H1 = hp.tile([ZROWS, T], F32, tag="H")
                    nc.scalar.activation(H1[:], Z1[0:ZROWS, :], AF.Prelu,
                                         bias=0.0, scale=1.0, alpha=SLOPE)
                    Hs = hsp.tile([ZROWS, T], F32, tag="Hs")
                    nc.vector.tensor_add(Hs[:], H0[:], H1[:])
                    pend.append((j, Hs))
                    if len(pend) > 3:
                        mm2_stage(*pend.pop(0))
                    inject(j)
                while pend:
                    mm2_stage(*pend.pop(0))

            def emit_gn(Fs2, E, etb, i, b):
                """Emit gather+sq now; return dict j -> [callback] with the
                rest of the GN tail spread across the partner block's tile
                stream (each cross-engine hop gets ~2 tiles of slack)."""
                F0, F1 = Fs2
                if BENCH_SKIP_GN:
                    return {}
                # gather split across SWDGE + sync rings (per-instruction
                # DMA transfers parallelize across rings)
                stage = gp.tile([SROWS, T], F32, tag="stage", name="stage")
                nc.gpsimd.dma_start(out=stage[0:SROWS // 2, :],
                                    in_=etb[0:D // 2, :])
                nc.sync.dma_start(out=stage[SROWS // 2:SROWS, :],
                                  in_=etb[D // 2:D, :])
                st = {}

                def p_sq():
                    st["sq"] = gp.tile([SROWS, T], F32, tag="sq", name="sq")
                    nc.vector.tensor_mul(st["sq"][:], stage[:], stage[:])

                def p_var():
                    st["var"] = sp.tile([SROWS, T], F32, tag="var",
                                        name="var")
                    nc.tensor.matmul(st["var"][0:SROWS, :], g1[:],
                                     st["sq"][:], start=True, stop=True)

                def p_rstd():
                    st["rstd"] = gp.tile([SROWS, T], F32, tag="rstd",
                                         name="rstd")
                    nc.scalar.activation(st["rstd"][:], st["var"][0:SROWS, :],
                                         AF.Abs_reciprocal_sqrt,
                                         bias=eps[0:SROWS, 0:1], scale=1.0)

                def p_y():
                    st["y"] = gp.tile([SROWS, T], F32, tag="y", name="y")
                    nc.vector.tensor_mul(st["y"][:], stage[:], st["rstd"][:])

                def p_upd():
                    st["upd"] = gp.tile([SROWS, T], F32, tag="upd",
                                        name="upd")
                    nc.scalar.activation(st["upd"][:], st["y"][:], AF.Prelu,
                                         bias=cg[:, 3 + i:4 + i],
                                         scale=cg[:, i:i + 1], alpha=SLOPE)

                def p_acc():
                    if i == 0:
                        nc.gpsimd.tensor_scalar(E[:], st["upd"][:], 1.0,
                                                None, OP.add)
                    else:
                        nc.gpsimd.tensor_add(E[:], E[:], st["upd"][:])

                def p_out():
                    if i < N_LAYERS - 1:
                        # Ehi cast on the idle GPSIMD - ScalarE is the
                        # binding pipeline engine
                        Ehi = gp.tile([SROWS, T], F16, tag="Ehi", name="Ehi")
                        nc.gpsimd.tensor_copy(Ehi[:], E[:])
                        Elo = gp.tile([SROWS, T], F16, tag="Elo", name="Elo")
                        nc.vector.tensor_sub(Elo[:], E[:], Ehi[:])
                        # emb refresh into both halves (c-major flat match),
                        # each split into row-halves on different rings so
                        # the transfers parallelize
                        h = SROWS // 2
                        nc.sync.dma_start(out=F0[113:116, :], in_=Ehi[0:h, :])
                        nc.gpsimd.dma_start(out=F0[116:119, :],
                                            in_=Ehi[h:SROWS, :])
                        nc.scalar.dma_start(out=F1[113:116, :],
                                            in_=Ehi[0:h, :])
                        nc.gpsimd.dma_start(out=F1[116:119, :],
                                            in_=Ehi[h:SROWS, :])
                        nc.sync.dma_start(out=F0[119:122, :], in_=Elo[0:h, :])
                        nc.scalar.dma_start(out=F0[122:125, :],
                                            in_=Elo[h:SROWS, :])
                        nc.scalar.dma_start(out=F1[119:122, :],
                                            in_=Elo[0:h, :])
                        nc.sync.dma_start(out=F1[122:125, :],
                                          in_=Elo[h:SROWS, :])
                    else:
                        h = SROWS // 2
                        sl = slice(b * W, (b + 1) * W)
                        nc.gpsimd.dma_start(out=out_e[0:D // 2, sl],
                                            in_=E[0:h, :])
                        nc.scalar.dma_start(out=out_e[D // 2:D, sl],
                                            in_=E[h:SROWS, :])

                return {0: [p_sq], 2: [p_var], 4: [p_rstd], 6: [p_y],
                        8: [p_upd], 10: [p_acc], 12: [p_out]}

            # Software-pipelined emission in even/odd pairs; each batch's
            # GN tail is deferred into the partner's tile stream.
            pairs = [(0, 1), (2, 3), (4, 5), (6, None)]
            Fs = {0: emit_load(0), 1: emit_load(1)}
            Es = {}
            tail = [None]

            def run_block(x, i, defer):
                etb = ep.tile([D, W], F32, tag="et", name="etb")
                emit_tiles(Fs[x], etb, i, after2=tail[0])
                tail[0] = None
                if i == N_LAYERS - 1 and x + 2 <= NB - 1:
                    Fs[x + 2] = emit_load(x + 2)
                fin = emit_gn(Fs[x], Es[x], etb, i, x)
                if defer:
                    tail[0] = fin
                else:
                    # lone block: its own next layer reads the refreshed
                    # emb rows, so the tail cannot be deferred
                    for j in sorted(fin):
                        for cb in fin[j]:
                            cb()

            for a, b in pairs:
                Es[a] = gp.tile([SROWS, T], F32, tag="E", name="E")
                if b is not None:
                    Es[b] = gp.tile([SROWS, T], F32, tag="E", name="E")
                for i in range(N_LAYERS):
                    for x in ((a, b) if b is not None else (a,)):
                        run_block(x, i, defer=(b is not None))

    print(f"[kernel] trace done {_time.time():.1f}", flush=True)
    nc.compile()
    print(f"[kernel] bacc compile done {_time.time():.1f}", flush=True)
    return nc


_RUNNER_CACHE = None


def get_runner():
    """Build (once) the jitted 8-core executable."""
    global _RUNNER_CACHE
    if _RUNNER_CACHE is not None:
        return _RUNNER_CACHE
    import jax
    import numpy as _np
    from jax.sharding import Mesh, PartitionSpec
    from jax.experimental.shard_map import shard_map
    import concourse.mybir as mybir
    from concourse.bass2jax import (
        install_neuronx_cc_hook, _bass_exec_p, partition_id_tensor)

    nc = _build_nc()
    install_neuronx_cc_hook()
    partition_name = nc.partition_id_tensor.name if nc.partition_id_tensor else None
    in_names, out_names, out_avals, zero_outs = [], [], [], []
    for alloc in nc.m.functions[0].allocations:
        if not isinstance(alloc, mybir.MemoryLocationSet):
            continue
        name = alloc.memorylocations[0].name
        if alloc.kind == "ExternalInput":
            if name != partition_name:
                in_names.append(name)
        elif alloc.kind == "ExternalOutput":
            out_names.append(name)
            shape = tuple(alloc.tensor_shape)
            dtype = mybir.dt.np(alloc.dtype)
            out_avals.append(jax.core.ShapedArray(shape, dtype))
            zero_outs.append(_np.zeros(shape, dtype))
    n_params = len(in_names)
    all_in_names = in_names + out_names
    if partition_name is not None:
        all_in_names.append(partition_name)

    def _body(*args):
        operands = list(args)
        if partition_name is not None:
            operands.append(partition_id_tensor())
        return tuple(_bass_exec_p.bind(
            *operands,
            out_avals=tuple(out_avals),
            in_names=tuple(all_in_names),
            out_names=tuple(out_names),
            lowering_input_output_aliases=(),
            sim_require_finite=True,
            sim_require_nnan=True,
            nc=nc,
        ))

    devices = jax.devices()[:N_CORES]
    mesh = Mesh(_np.asarray(devices), ("core",))
    nin = n_params + len(out_names)
    sharded = jax.jit(shard_map(
        _body, mesh=mesh,
        in_specs=(PartitionSpec("core"),) * nin,
        out_specs=(PartitionSpec("core"),) * len(out_names),
        check_rep=False))
    _RUNNER_CACHE = {
        "fn": sharded, "in_names": in_names, "out_names": out_names,
        "zero_outs": zero_outs, "mesh": mesh, "nc": nc,
    }
    return _RUNNER_CACHE


def run_cores(in_maps):
    """Run the 8-core kernel on a list of per-core input dicts."""
    import numpy as _np
    r = get_runner()
    concat_in = [
        _np.concatenate([in_maps[c][name] for c in range(N_CORES)], axis=0)
        for name in r["in_names"]
    ] + [_np.concatenate([z] * N_CORES, axis=0) for z in r["zero_outs"]]
    outs = r["fn"](*concat_in)
    res = []
    for c in range(N_CORES):
        d = {}
        for i, name in enumerate(r["out_names"]):
            full = _np.asarray(outs[i])
            rows = full.shape[0] // N_CORES
            d[name] = full[c * rows:(c + 1) * rows]
        res.append(d)
    return res


def make_in_maps(dist, atomtypes, W1, b1, W2, b2, gw, gb):
    dist = np.asarray(dist, np.float32)
    atomtypes = np.asarray(atomtypes, np.float32)
    B, N, _, _ = atomtypes.shape
    P = B * N
    assert P == N_CORES * PC

    l1, l2, G1, cstg = _pack_weights(
        np.asarray(W1, np.float32), np.asarray(b1, np.float32),
        np.asarray(W2, np.float32), np.asarray(b2, np.float32),
        np.asarray(gw, np.float32), np.asarray(gb, np.float32))

    atom = atomtypes.reshape(P, K, D)
    dst = dist.reshape(P, K)
    a_hi = np.float16(atom)
    a_lo = np.float16(atom - a_hi.astype(np.float32))
    d_hi = np.float16(dst)
    d_lo = np.float16(dst - d_hi.astype(np.float32))

    in_maps = []
    for c in range(N_CORES):
        sl = slice(c * PC, (c + 1) * PC)
        ads = []
        for h in range(2):
            ks = slice(h * 8, h * 8 + 8)
            a = np.zeros((PP, AD_ROWS), np.float16)
            a[:PC, 0:48] = a_hi[sl, ks].reshape(PC, 48)
            a[:PC, 48:96] = a_lo[sl, ks].reshape(PC, 48)
            a[:PC, 96:104] = d_hi[sl, ks]
            a[:PC, 104:112] = d_lo[sl, ks]
            a[:, 112] = np.float16(1.0)
            ads.append(np.ascontiguousarray(a.T))
        in_maps.append({
            "ad0": ads[0], "ad1": ads[1],
            "lhsT1": l1, "lhsT2": l2, "g1": G1, "cstg": cstg,
        })
    return in_maps, (B, N)


def kernel(dist, atomtypes, W1, b1, W2, b2, gw, gb):
    in_maps, (B, N) = make_in_maps(dist, atomtypes, W1, b1, W2, b2, gw, gb)
    res = run_cores(in_maps)
    outs = [res[c]["out"][:, :PC].T for c in range(N_CORES)]
    return np.concatenate(outs, axis=0).reshape(B, N, D).astype(np.float32)


if __name__ == "__main__":
    rng = np.random.default_rng(0)
    inputs = {
        "dist": rng.random((4, 100000, 16, 1), dtype=np.float32),
        "atomtypes": rng.random((4, 100000, 16, 6), dtype=np.float32),
        "W1": rng.random((3, 13, 13), dtype=np.float32) - 0.5,
        "b1": rng.random((3, 13), dtype=np.float32) - 0.5,
        "W2": rng.random((3, 13, 6), dtype=np.float32) - 0.5,
        "b2": rng.random((3, 6), dtype=np.float32) - 0.5,
        "gw": np.ones((3, 6), np.float32),
        "gb": np.zeros((3, 6), np.float32),
    }
    out = kernel(**inputs)
    print(out.shape, out.dtype)
